# revision 18
# baseline (speedup 1.0000x reference)
"""AnemllQATLinear Trainium2 kernel (8 NeuronCores, column-parallel).

y = x @ fake_quant(weight).T + bias + lora_scaling * (x @ lora_A.T) @ lora_B.T

Strategy:
  - Shard out_features (O=4096) across 8 cores (512 each). Replicate x.
  - Host prep: full fake-quantization of the weight (clip/round/LUT/rescale)
    into bf16 wq^T shards [I, 512]; x -> x^T as bf16 [I, N] (shared).
  - Device per core: pure dense GEMM y[N, 512] = (x^T).T @ wq^T with fused
    bias add on PSUM eviction. m-tiles of 512 rows ping-pong between two
    4-bank PSUM halves so eviction (DVE bias-add + DMA out) fully overlaps
    the next m-tile's matmuls and the PE never idles.
  - LoRA is zero in the common case (lora_B == 0); host fallback otherwise.
  - Host gathers per-core y slices -> full [4, 4096, 4096] f32.
"""
import sys
import types
from contextlib import ExitStack

import numpy as np
import ml_dtypes

import concourse.bass as bass
import concourse.mybir as mybir
import concourse.tile as tile
from concourse import bacc
from concourse.bass_utils import run_bass_kernel_spmd

P = 128
N_CORES = 8
O_FULL = 4096
O_LOC = O_FULL // N_CORES  # 512
I_DIM = 4096               # contraction dim K
B, S = 4, 4096
N_ROWS = B * S             # 16384
GS = 128                   # quant group size
G = I_DIM // GS            # 32 groups
EPS = 1e-8
LUT_SIZE = 16
LORA_SCALING = 2.0
QSTEP = 2.0 / (LUT_SIZE - 1)

F32 = mybir.dt.float32
BF16 = mybir.dt.bfloat16
ALU = mybir.AluOpType

K_TILE = 512
K_TILES_N = I_DIM // K_TILE  # 8
K_SUB = K_TILE // P          # 4
M_TILE = 512
M_TILES = N_ROWS // M_TILE   # 32
M_SUB = M_TILE // P          # 4


def _install_ntff_hook():
    """Enable trace=True under axon: bass_utils needs antenv.axon_hooks."""
    try:
        import antenv

        if "antenv.axon_hooks" not in sys.modules:
            mod = types.ModuleType("antenv.axon_hooks")
            mod._hook = None
            mod.set_axon_ntff_profile_hook = lambda h: setattr(mod, "_hook", h)
            mod.get_axon_ntff_profile_hook = lambda: mod._hook
            sys.modules["antenv.axon_hooks"] = mod
            antenv.axon_hooks = mod
        from trn_agent_boot.trn_boot import _ntff_profile_via_ctypes

        sys.modules["antenv.axon_hooks"].set_axon_ntff_profile_hook(
            _ntff_profile_via_ctypes("/opt/axon/libaxon_pjrt.so")
        )
        import concourse.bass_utils as bass_utils

        bass_utils.upload_artifacts = lambda tmpdir: str(tmpdir)
    except Exception:
        pass


def build_nc():
    nc = bacc.Bacc("TRN2", target_bir_lowering=False, debug=False, num_devices=N_CORES)

    xt = nc.dram_tensor("xt", [I_DIM, N_ROWS], BF16, kind="ExternalInput")
    wqt = nc.dram_tensor("wqt", [I_DIM, O_LOC], BF16, kind="ExternalInput")
    bias_in = nc.dram_tensor("biasv", [1, O_LOC], F32, kind="ExternalInput")
    y = nc.dram_tensor("y", [N_ROWS, O_LOC], F32, kind="ExternalOutput")

    with ExitStack() as ctx:
        tc = ctx.enter_context(tile.TileContext(nc))
        constp = ctx.enter_context(tc.tile_pool(name="const", bufs=1))
        wq_pool = ctx.enter_context(tc.tile_pool(name="wq_pool", bufs=1))
        kxm_pool = ctx.enter_context(tc.tile_pool(name="kxm_pool", bufs=24))
        psum_pool = ctx.enter_context(
            tc.tile_pool(name="psum_pool", bufs=1, space="PSUM"))
        ypool = ctx.enter_context(tc.tile_pool(name="ypool", bufs=8))

        # bias broadcast to all partitions once (SWDGE; off the main queue)
        bias_bc = constp.tile([P, O_LOC], F32)
        nc.gpsimd.dma_start(out=bias_bc[:], in_=bias_in[:].broadcast_to([P, O_LOC]))

        # Dummy warm-up matmuls on scratch SBUF: keep the PE busy during the
        # input-DMA ramp so the HAM clock gate opens (1.2 -> 2.4 GHz) before
        # the first real matmul. They write the mt-parity-1 PSUM half, whose
        # first real use (mt=1, start=True) overwrites the bank.
        scratch = constp.tile([P, 128 + O_LOC], BF16)
        nc.vector.memset(scratch[:], 0.0)
        warm_ps = [psum_pool.tile([P, O_LOC], F32, tag=f"ps1_{j}",
                                  name=f"warm_ps{j}") for j in range(M_SUB)]
        for w in range(12):
            nc.tensor.matmul(
                warm_ps[w % M_SUB][:], scratch[:, 0:128], scratch[:, 128:],
                start=True, stop=True)

        xv = xt[:].rearrange("(po pi) f -> pi po f", pi=P)   # [128, G, N_ROWS]
        wv = wqt[:].rearrange("(po pi) f -> pi po f", pi=P)  # [128, G, O_LOC]

        # wq^T SBUF-resident, interleaved with the first m-tile's x tiles so
        # the first matmuls can start as soon as wq[0] + kxm[0,0] land.
        # The k=0 tiles are split per-ki (128 KB grains) to cut time-to-first-MM.
        wq_ref = {}   # (k, ki) -> AP [P, O_LOC]
        x_ref = {}    # (mt, k, ki) -> AP [P, M_TILE]
        kxm_tiles = {}
        for ki in range(K_SUB):
            t = kxm_pool.tile([P, 1, M_TILE], BF16, tag="kxmf", name=f"kxmf{ki}")
            nc.sync.dma_start(out=t[:], in_=xv[:, ki:ki + 1, 0:M_TILE])
            x_ref[(0, 0, ki)] = t[:, 0, :]
            wt = wq_pool.tile([P, 1, O_LOC], BF16, tag=f"wqf{ki}", name=f"wqf{ki}")
            nc.scalar.dma_start(out=wt[:], in_=wv[:, ki:ki + 1, :])
            wq_ref[(0, ki)] = wt[:, 0, :]
        for k in range(1, K_TILES_N):
            wt = wq_pool.tile([P, K_SUB, O_LOC], BF16, tag=f"wqt{k}", name=f"wqt{k}")
            nc.scalar.dma_start(out=wt[:], in_=wv[:, k * K_SUB:(k + 1) * K_SUB, :])
            for ki in range(K_SUB):
                wq_ref[(k, ki)] = wt[:, ki, :]
            t = kxm_pool.tile([P, K_SUB, M_TILE], BF16, tag="kxm", name=f"kxm_0_{k}")
            eng = nc.scalar if k % 2 == 1 else nc.sync
            eng.dma_start(
                out=t[:], in_=xv[:, k * K_SUB:(k + 1) * K_SUB, 0:M_TILE])
            kxm_tiles[(0, k)] = t

        yv = y[:].rearrange("(po pi) f -> pi po f", pi=P)    # [128, N/128, O_LOC]

        def evict(mt, j, ps_j, split=1, eng=None):
            eng = eng or nc.scalar
            w = O_LOC // split
            for h in range(split):
                yt = ypool.tile([P, w], F32, tag=f"yt{h}" if split > 1 else "yt",
                                name=f"yt{mt}_{j}_{h}")
                nc.vector.tensor_tensor(
                    out=yt[:], in0=ps_j[:, h * w:(h + 1) * w],
                    in1=bias_bc[:, h * w:(h + 1) * w], op=ALU.add)
                eng.dma_start(
                    out=yv[:, mt * M_SUB + j, h * w:(h + 1) * w], in_=yt[:])

        def issue_x(mt, split_queues=False):
            for k in range(K_TILES_N):
                t = kxm_pool.tile(
                    [P, K_SUB, M_TILE], BF16, tag="kxm", name=f"kxm_{mt}_{k}")
                eng = nc.scalar if (split_queues and k % 2 == 1) else nc.sync
                eng.dma_start(
                    out=t[:],
                    in_=xv[:, k * K_SUB:(k + 1) * K_SUB,
                           mt * M_TILE:(mt + 1) * M_TILE])
                kxm_tiles[(mt, k)] = t

        issue_x(1, split_queues=True)
        for mt in range(M_TILES):
            if mt + 2 < M_TILES:
                issue_x(mt + 2)
            for k in range(K_TILES_N):
                if (mt, k) in kxm_tiles:
                    xt_ = kxm_tiles.pop((mt, k))
                    for ki in range(K_SUB):
                        x_ref[(mt, k, ki)] = xt_[:, ki, :]

            half = mt % 2
            ps = [psum_pool.tile([P, O_LOC], F32, tag=f"ps{half}_{j}",
                                 name=f"ps{half}_{j}_{mt}")
                  for j in range(M_SUB)]
            if mt < M_TILES - 1:
                # k-major, j-inner: all four chains advance together
                for k in range(K_TILES_N):
                    for ki in range(K_SUB):
                        rhs = wq_ref[(k, ki)]
                        for j in range(M_SUB):
                            nc.tensor.matmul(
                                ps[j][:],
                                x_ref[(mt, k, ki)][:, bass.ts(j, P)],
                                rhs,
                                start=(k == 0 and ki == 0),
                                stop=(k == K_TILES_N - 1 and ki == K_SUB - 1),
                            )
                for j in range(M_SUB):
                    evict(mt, j, ps[j])
            else:
                # last m-tile: j-outer so each chain finishes (and evicts)
                # while the next chain is still on the PE -> short tail
                for j in range(M_SUB):
                    for k in range(K_TILES_N):
                        for ki in range(K_SUB):
                            nc.tensor.matmul(
                                ps[j][:],
                                x_ref[(mt, k, ki)][:, bass.ts(j, P)],
                                wq_ref[(k, ki)],
                                start=(k == 0 and ki == 0),
                                stop=(k == K_TILES_N - 1 and ki == K_SUB - 1),
                            )
                    evict(mt, j, ps[j], split=2 if j == M_SUB - 1 else 1,
                          eng=nc.sync if j >= M_SUB - 2 else None)

    nc.compile()
    return nc


_NC_CACHE: dict = {}


def _get_nc():
    if "nc" not in _NC_CACHE:
        _NC_CACHE["nc"] = build_nc()
    return _NC_CACHE["nc"]


def kernel(x, weight, bias, scale_A, scale_B, lut, lora_A, lora_B, **_):
    _install_ntff_hook()

    x = np.asarray(x, dtype=np.float32)
    weight = np.asarray(weight, dtype=np.float32)
    bias = np.asarray(bias, dtype=np.float32)
    scale_A = np.asarray(scale_A, dtype=np.float32)
    scale_B = np.asarray(scale_B, dtype=np.float32)
    lut = np.asarray(lut, dtype=np.float32)
    lora_A = np.asarray(lora_A, dtype=np.float32)
    lora_B = np.asarray(lora_B, dtype=np.float32)

    # ---- host prep: full fake-quantization, exactly as the reference ----
    s_full = np.maximum(scale_A @ scale_B, EPS)              # [O, G]
    grouped = weight.reshape(O_FULL, G, GS)
    normalized = np.clip(grouped / s_full[:, :, None], -1.0, 1.0)
    idx = np.clip(np.round((normalized + 1.0) / QSTEP).astype(np.int32),
                  0, LUT_SIZE - 1)
    wq = (lut[idx] * s_full[:, :, None]).reshape(O_FULL, I_DIM)

    x2 = x.reshape(N_ROWS, I_DIM)
    xt_bf16 = np.ascontiguousarray(x2.astype(ml_dtypes.bfloat16).T)  # [I, N]
    wqt_bf16 = np.ascontiguousarray(wq.astype(ml_dtypes.bfloat16).T)  # [I, O]

    in_maps = []
    for c in range(N_CORES):
        sl = slice(c * O_LOC, (c + 1) * O_LOC)
        in_maps.append({
            "xt": xt_bf16,
            "wqt": np.ascontiguousarray(wqt_bf16[:, sl]),
            "biasv": bias[sl].reshape(1, O_LOC).copy(),
        })

    nc = _get_nc()
    res = run_bass_kernel_spmd(
        nc, in_maps, core_ids=list(range(N_CORES)), trace=False
    )
    global LAST_RESULT
    LAST_RESULT = res

    y = np.concatenate([res.results[c]["y"] for c in range(N_CORES)], axis=1)
    # host-side correction for the rare nonzero-LoRA path (rank 16)
    if np.any(lora_B != 0.0):
        y = y + (x2 @ lora_A.T) @ (LORA_SCALING * lora_B.T)
    return np.ascontiguousarray(y.reshape(B, S, O_FULL).astype(np.float32))


if __name__ == "__main__":
    rng = np.random.default_rng(0)
    x = rng.standard_normal((B, S, I_DIM), dtype=np.float32)
    weight = (rng.standard_normal((O_FULL, I_DIM), dtype=np.float32) * 0.02)
    bias = rng.uniform(-0.015, 0.015, O_FULL).astype(np.float32)
    sf = np.maximum(np.abs(weight.reshape(O_FULL, G, GS)).max(axis=2), EPS)
    u, s, vh = np.linalg.svd(sf, full_matrices=False)
    scale_A = (u[:, :4] * s[:4]).astype(np.float32)
    scale_B = vh[:4, :].astype(np.float32)
    lut = np.linspace(-1, 1, LUT_SIZE, dtype=np.float32)
    lora_A = rng.standard_normal((16, I_DIM), dtype=np.float32) * 0.02
    lora_B = np.zeros((O_FULL, 16), dtype=np.float32)
    y = kernel(x=x, weight=weight, bias=bias, scale_A=scale_A, scale_B=scale_B,
               lut=lut, lora_A=lora_A, lora_B=lora_B)
    print("kernel output:", y.shape, y.dtype)


# revision 20
# speedup vs baseline: 1.0399x; 1.0399x over previous
"""AnemllQATLinear Trainium2 kernel (8 NeuronCores, column-parallel).

y = x @ fake_quant(weight).T + bias + lora_scaling * (x @ lora_A.T) @ lora_B.T

Strategy:
  - Shard out_features (O=4096) across 8 cores (512 each). Replicate x.
  - Host prep: full fake-quantization of the weight (clip/round/LUT/rescale)
    into bf16 wq^T shards [I, 512]; x -> x^T as bf16 [I, N] (shared).
  - Device per core: pure dense GEMM y[N, 512] = (x^T).T @ wq^T with fused
    bias add on PSUM eviction. m-tiles of 512 rows ping-pong between two
    4-bank PSUM halves so eviction (DVE bias-add + DMA out) fully overlaps
    the next m-tile's matmuls and the PE never idles.
  - LoRA is zero in the common case (lora_B == 0); host fallback otherwise.
  - Host gathers per-core y slices -> full [4, 4096, 4096] f32.
"""
import sys
import types
from contextlib import ExitStack

import numpy as np
import ml_dtypes

import concourse.bass as bass
import concourse.mybir as mybir
import concourse.tile as tile
from concourse import bacc
from concourse.bass_utils import run_bass_kernel_spmd

P = 128
N_CORES = 8
O_FULL = 4096
O_LOC = O_FULL // N_CORES  # 512
I_DIM = 4096               # contraction dim K
B, S = 4, 4096
N_ROWS = B * S             # 16384
GS = 128                   # quant group size
G = I_DIM // GS            # 32 groups
EPS = 1e-8
LUT_SIZE = 16
LORA_SCALING = 2.0
QSTEP = 2.0 / (LUT_SIZE - 1)

F32 = mybir.dt.float32
BF16 = mybir.dt.bfloat16
ALU = mybir.AluOpType

K_TILE = 512
K_TILES_N = I_DIM // K_TILE  # 8
K_SUB = K_TILE // P          # 4
M_TILE = 512
M_TILES = N_ROWS // M_TILE   # 32
M_SUB = M_TILE // P          # 4


def _install_ntff_hook():
    """Enable trace=True under axon: bass_utils needs antenv.axon_hooks."""
    try:
        import antenv

        if "antenv.axon_hooks" not in sys.modules:
            mod = types.ModuleType("antenv.axon_hooks")
            mod._hook = None
            mod.set_axon_ntff_profile_hook = lambda h: setattr(mod, "_hook", h)
            mod.get_axon_ntff_profile_hook = lambda: mod._hook
            sys.modules["antenv.axon_hooks"] = mod
            antenv.axon_hooks = mod
        from trn_agent_boot.trn_boot import _ntff_profile_via_ctypes

        sys.modules["antenv.axon_hooks"].set_axon_ntff_profile_hook(
            _ntff_profile_via_ctypes("/opt/axon/libaxon_pjrt.so")
        )
        import concourse.bass_utils as bass_utils

        bass_utils.upload_artifacts = lambda tmpdir: str(tmpdir)
    except Exception:
        pass


def build_nc():
    nc = bacc.Bacc("TRN2", target_bir_lowering=False, debug=False, num_devices=N_CORES)

    xt = nc.dram_tensor("xt", [I_DIM, N_ROWS], BF16, kind="ExternalInput")
    wqt = nc.dram_tensor("wqt", [I_DIM, O_LOC], BF16, kind="ExternalInput")
    bias_in = nc.dram_tensor("biasv", [1, O_LOC], F32, kind="ExternalInput")
    y = nc.dram_tensor("y", [N_ROWS, O_LOC], F32, kind="ExternalOutput")

    with ExitStack() as ctx:
        tc = ctx.enter_context(tile.TileContext(nc))
        constp = ctx.enter_context(tc.tile_pool(name="const", bufs=1))
        wq_pool = ctx.enter_context(tc.tile_pool(name="wq_pool", bufs=1))
        kxm_pool = ctx.enter_context(tc.tile_pool(name="kxm_pool", bufs=24))
        psum_pool = ctx.enter_context(
            tc.tile_pool(name="psum_pool", bufs=1, space="PSUM"))
        ypool = ctx.enter_context(tc.tile_pool(name="ypool", bufs=8))

        # bias broadcast to all partitions once (SWDGE; off the main queue)
        bias_bc = constp.tile([P, O_LOC], F32)
        nc.gpsimd.dma_start(out=bias_bc[:], in_=bias_in[:].broadcast_to([P, O_LOC]))

        # Dummy warm-up matmuls on scratch SBUF: keep the PE busy during the
        # input-DMA ramp so the HAM clock gate opens (1.2 -> 2.4 GHz) before
        # the first real matmul. They write the mt-parity-1 PSUM half, whose
        # first real use (mt=1, start=True) overwrites the bank.
        scratch = constp.tile([P, 128 + O_LOC], BF16)
        nc.vector.memset(scratch[:], 0.0)
        warm_ps = [psum_pool.tile([P, O_LOC], F32, tag=f"ps1_{j}",
                                  name=f"warm_ps{j}") for j in range(M_SUB)]
        for w in range(12):
            nc.tensor.matmul(
                warm_ps[w % M_SUB][:], scratch[:, 0:128], scratch[:, 128:],
                start=True, stop=True)

        xv = xt[:].rearrange("(po pi) f -> pi po f", pi=P)   # [128, G, N_ROWS]
        wv = wqt[:].rearrange("(po pi) f -> pi po f", pi=P)  # [128, G, O_LOC]

        # wq^T SBUF-resident, interleaved with the first m-tile's x tiles so
        # the first matmuls can start as soon as wq[0] + kxm[0,0] land.
        # The k=0 tiles are split per-ki (128 KB grains) to cut time-to-first-MM.
        wq_ref = {}   # (k, ki) -> AP [P, O_LOC]
        x_ref = {}    # (mt, k, ki) -> AP [P, M_TILE]
        kxm_tiles = {}
        for ki in range(K_SUB):
            t = kxm_pool.tile([P, 1, M_TILE], BF16, tag="kxmf", name=f"kxmf{ki}")
            nc.sync.dma_start(out=t[:], in_=xv[:, ki:ki + 1, 0:M_TILE])
            x_ref[(0, 0, ki)] = t[:, 0, :]
            wt = wq_pool.tile([P, 1, O_LOC], BF16, tag=f"wqf{ki}", name=f"wqf{ki}")
            nc.scalar.dma_start(out=wt[:], in_=wv[:, ki:ki + 1, :])
            wq_ref[(0, ki)] = wt[:, 0, :]
        for k in range(1, K_TILES_N):
            wt = wq_pool.tile([P, K_SUB, O_LOC], BF16, tag=f"wqt{k}", name=f"wqt{k}")
            nc.scalar.dma_start(out=wt[:], in_=wv[:, k * K_SUB:(k + 1) * K_SUB, :])
            for ki in range(K_SUB):
                wq_ref[(k, ki)] = wt[:, ki, :]
            t = kxm_pool.tile([P, K_SUB, M_TILE], BF16, tag="kxm", name=f"kxm_0_{k}")
            nc.sync.dma_start(
                out=t[:], in_=xv[:, k * K_SUB:(k + 1) * K_SUB, 0:M_TILE])
            kxm_tiles[(0, k)] = t

        yv = y[:].rearrange("(po pi) f -> pi po f", pi=P)    # [128, N/128, O_LOC]

        def evict(mt, j, ps_j, split=1, eng=None):
            eng = eng or nc.scalar
            w = O_LOC // split
            for h in range(split):
                yt = ypool.tile([P, w], F32, tag=f"yt{h}" if split > 1 else "yt",
                                name=f"yt{mt}_{j}_{h}")
                nc.vector.tensor_tensor(
                    out=yt[:], in0=ps_j[:, h * w:(h + 1) * w],
                    in1=bias_bc[:, h * w:(h + 1) * w], op=ALU.add)
                eng.dma_start(
                    out=yv[:, mt * M_SUB + j, h * w:(h + 1) * w], in_=yt[:])

        def issue_x(mt):
            for k in range(K_TILES_N):
                t = kxm_pool.tile(
                    [P, K_SUB, M_TILE], BF16, tag="kxm", name=f"kxm_{mt}_{k}")
                nc.sync.dma_start(
                    out=t[:],
                    in_=xv[:, k * K_SUB:(k + 1) * K_SUB,
                           mt * M_TILE:(mt + 1) * M_TILE])
                kxm_tiles[(mt, k)] = t

        issue_x(1)
        for mt in range(M_TILES):
            if mt + 2 < M_TILES:
                issue_x(mt + 2)
            for k in range(K_TILES_N):
                if (mt, k) in kxm_tiles:
                    xt_ = kxm_tiles.pop((mt, k))
                    for ki in range(K_SUB):
                        x_ref[(mt, k, ki)] = xt_[:, ki, :]

            half = mt % 2
            ps = [psum_pool.tile([P, O_LOC], F32, tag=f"ps{half}_{j}",
                                 name=f"ps{half}_{j}_{mt}")
                  for j in range(M_SUB)]
            if mt < M_TILES - 1:
                # k-major, j-inner: all four chains advance together
                for k in range(K_TILES_N):
                    for ki in range(K_SUB):
                        rhs = wq_ref[(k, ki)]
                        for j in range(M_SUB):
                            nc.tensor.matmul(
                                ps[j][:],
                                x_ref[(mt, k, ki)][:, bass.ts(j, P)],
                                rhs,
                                start=(k == 0 and ki == 0),
                                stop=(k == K_TILES_N - 1 and ki == K_SUB - 1),
                            )
                for j in range(M_SUB):
                    evict(mt, j, ps[j])
            else:
                # last m-tile: j-outer so each chain finishes (and evicts)
                # while the next chain is still on the PE -> short tail
                for j in range(M_SUB):
                    for k in range(K_TILES_N):
                        for ki in range(K_SUB):
                            nc.tensor.matmul(
                                ps[j][:],
                                x_ref[(mt, k, ki)][:, bass.ts(j, P)],
                                wq_ref[(k, ki)],
                                start=(k == 0 and ki == 0),
                                stop=(k == K_TILES_N - 1 and ki == K_SUB - 1),
                            )
                    evict(mt, j, ps[j], split=2 if j == M_SUB - 1 else 1,
                          eng=nc.sync if j >= M_SUB - 2 else None)

    nc.compile()
    return nc


_NC_CACHE: dict = {}


def _get_nc():
    if "nc" not in _NC_CACHE:
        _NC_CACHE["nc"] = build_nc()
    return _NC_CACHE["nc"]


def kernel(x, weight, bias, scale_A, scale_B, lut, lora_A, lora_B, **_):
    _install_ntff_hook()

    x = np.asarray(x, dtype=np.float32)
    weight = np.asarray(weight, dtype=np.float32)
    bias = np.asarray(bias, dtype=np.float32)
    scale_A = np.asarray(scale_A, dtype=np.float32)
    scale_B = np.asarray(scale_B, dtype=np.float32)
    lut = np.asarray(lut, dtype=np.float32)
    lora_A = np.asarray(lora_A, dtype=np.float32)
    lora_B = np.asarray(lora_B, dtype=np.float32)

    # ---- host prep: full fake-quantization, exactly as the reference ----
    s_full = np.maximum(scale_A @ scale_B, EPS)              # [O, G]
    grouped = weight.reshape(O_FULL, G, GS)
    normalized = np.clip(grouped / s_full[:, :, None], -1.0, 1.0)
    idx = np.clip(np.round((normalized + 1.0) / QSTEP).astype(np.int32),
                  0, LUT_SIZE - 1)
    wq = (lut[idx] * s_full[:, :, None]).reshape(O_FULL, I_DIM)

    x2 = x.reshape(N_ROWS, I_DIM)
    xt_bf16 = np.ascontiguousarray(x2.astype(ml_dtypes.bfloat16).T)  # [I, N]
    wqt_bf16 = np.ascontiguousarray(wq.astype(ml_dtypes.bfloat16).T)  # [I, O]

    in_maps = []
    for c in range(N_CORES):
        sl = slice(c * O_LOC, (c + 1) * O_LOC)
        in_maps.append({
            "xt": xt_bf16,
            "wqt": np.ascontiguousarray(wqt_bf16[:, sl]),
            "biasv": bias[sl].reshape(1, O_LOC).copy(),
        })

    nc = _get_nc()
    res = run_bass_kernel_spmd(
        nc, in_maps, core_ids=list(range(N_CORES)), trace=False
    )
    global LAST_RESULT
    LAST_RESULT = res

    y = np.concatenate([res.results[c]["y"] for c in range(N_CORES)], axis=1)
    # host-side correction for the rare nonzero-LoRA path (rank 16)
    if np.any(lora_B != 0.0):
        y = y + (x2 @ lora_A.T) @ (LORA_SCALING * lora_B.T)
    return np.ascontiguousarray(y.reshape(B, S, O_FULL).astype(np.float32))


if __name__ == "__main__":
    rng = np.random.default_rng(0)
    x = rng.standard_normal((B, S, I_DIM), dtype=np.float32)
    weight = (rng.standard_normal((O_FULL, I_DIM), dtype=np.float32) * 0.02)
    bias = rng.uniform(-0.015, 0.015, O_FULL).astype(np.float32)
    sf = np.maximum(np.abs(weight.reshape(O_FULL, G, GS)).max(axis=2), EPS)
    u, s, vh = np.linalg.svd(sf, full_matrices=False)
    scale_A = (u[:, :4] * s[:4]).astype(np.float32)
    scale_B = vh[:4, :].astype(np.float32)
    lut = np.linspace(-1, 1, LUT_SIZE, dtype=np.float32)
    lora_A = rng.standard_normal((16, I_DIM), dtype=np.float32) * 0.02
    lora_B = np.zeros((O_FULL, 16), dtype=np.float32)
    y = kernel(x=x, weight=weight, bias=bias, scale_A=scale_A, scale_B=scale_B,
               lut=lut, lora_A=lora_A, lora_B=lora_B)
    print("kernel output:", y.shape, y.dtype)
